# revision 45
# baseline (speedup 1.0000x reference)
"""Trainium2 Bass kernel for nn_Attention (dual-key re-softmaxed attention).

Reference computation (per batch sample, N = H*W = 4096, C = 256, CQ = 64):
    q  = Wq  @ x3 + bq          (CQ, N)
    k1 = Wk  @ x1 + bk          (CQ, N)
    k2 = Wk2 @ x2 + bk2         (CQ, N)
    A1 = softmax_rows(q^T k1)   (N, N)
    A2 = softmax_rows(q^T k2)
    A  = softmax_rows(A1 + A2)                      -> output "attn"
    out  = gamma  * ((Wv  @ x3 + bv ) @ A^T) + x3   -> output "out"
    out2 = gamma2 * ((Wv2 @ xt + bv2) @ A^T) + xt   -> output "out2"

Sharding: 8 cores = 4 batch samples x 2 query-halves (2048 queries each).
Each core gets its sample's full x1/x2/x3/xt (for keys/values) plus the
query-slice of x3/xt (for q and the residuals), computes its (2048, 4096)
slice of A and (256, 2048) slices of out/out2.

Numerics:
  - softmax without max-subtraction: logits are ~N(0, 5^2) (|max| < ~35),
    exp stays far inside fp32 range; mathematically identical to the
    reference's max-subtracted softmax.
  - row sums come free from the Exp activation's accum_out.
  - QK^T matmuls default to bf16 hi/lo 3-pass (q and k each split into
    bf16 hi+lo; S = qh*kh + ql*kh + qh*kl accumulated in fp32 PSUM; only
    the ~2^-18 ql*kl term is dropped) -> ~2e-5 attn error. The first two
    terms are computed by ONE 128-deep matmul ([q_hi; q_lo] stacked on
    the partition axis against duplicated [k_hi; k_hi]), so the whole
    thing costs 2 cycles/row despite the 64-deep contraction.
  - AV matmuls in bf16: the AV result is scaled by gamma (~0.1) and added
    to the x3/xt residual which dominates by ~1000x, so bf16 error is
    attenuated far below fp32 epsilon there.
  - bias bv is folded into the residual (rows of A sum to 1), gamma is
    folded into V's weights host-side.

Schedule: per 128-query block, S-matmuls stream quarter-wise through 3
rotating 2-bank PSUM tiles into ACT Exp (ping-pong); A@V matmuls for block
i-3 are drained in small chunks between S-phase quarters so PE never
starves ACT; each block's exp3-and-later chain is emitted after the NEXT
block's S-phase (software pipelining, 5 E-tile slots); A^T comes from a
bf16 HWDGE DMA-transpose into a 3-deep ring.
"""

import os

import numpy as np
import ml_dtypes

import concourse.bass as bass
import concourse.mybir as mybir
import concourse.tile as tile
from concourse import bacc
from concourse.bass_utils import run_bass_kernel_spmd

F32 = mybir.dt.float32
F32R = mybir.dt.float32r
BF16 = mybir.dt.bfloat16

B, C, HH, WW = 4, 256, 64, 64
N = HH * WW          # 4096 keys / pixels
CQ = C // 4          # 64 query channels
NQ = N // 2          # 2048 queries per core
QB = 128             # queries per block (psum partition dim)
NBLK = NQ // QB      # 16 blocks per core
N_CORES = 8

# QK matmul mode:
#   "b3"   bf16 hi/lo 3-pass: S = qh*kh + ql*kh + qh*kl, error ~2e-5 (default)
#   "f32r" single-pass float32r, error ~1e-3
#   "f32"  exact fp32, 4 cycles/row on PE
QK_MODE = os.environ.get("ATTN_QK", "b3")


def build_nc(qk_mode: str = QK_MODE):
    qk_f32r = qk_mode == "f32r"
    qkd = F32R if qk_f32r else F32
    nc = bacc.Bacc("TRN2", target_bir_lowering=False, debug=True)

    # ---- DRAM I/O (per-core shapes) ----
    x1_d = nc.dram_tensor("x1", [C, N], F32, kind="ExternalInput")
    x2_d = nc.dram_tensor("x2", [C, N], F32, kind="ExternalInput")
    x3_d = nc.dram_tensor("x3", [C, N], F32, kind="ExternalInput")
    xt_d = nc.dram_tensor("xt", [C, N], F32, kind="ExternalInput")
    x3q_d = nc.dram_tensor("x3q", [C, NQ], F32, kind="ExternalInput")
    xtq_d = nc.dram_tensor("xtq", [C, NQ], F32, kind="ExternalInput")
    # host-pretransposed weights, tile layouts:
    #   wqt[p, j, o] = Wq[o, 128*j + p]
    wqt_d = nc.dram_tensor("wqt", [128, 2, CQ], F32, kind="ExternalInput")
    wkt_d = nc.dram_tensor("wkt", [128, 2, CQ], F32, kind="ExternalInput")
    wk2t_d = nc.dram_tensor("wk2t", [128, 2, CQ], F32, kind="ExternalInput")
    #   wvt[p, j, o] = gamma * Wv[o, 128*j + p]  (bf16)
    wvt_d = nc.dram_tensor("wvt", [128, 2, C], BF16, kind="ExternalInput")
    wv2t_d = nc.dram_tensor("wv2t", [128, 2, C], BF16, kind="ExternalInput")
    bq_d = nc.dram_tensor("bq", [CQ, 1], F32, kind="ExternalInput")
    bk_d = nc.dram_tensor("bk", [CQ, 1], F32, kind="ExternalInput")
    bk2_d = nc.dram_tensor("bk2", [CQ, 1], F32, kind="ExternalInput")
    #   gbv[p, c] = gamma * bv[128*c + p]
    gbv_d = nc.dram_tensor("gbv", [128, 2], F32, kind="ExternalInput")
    gbv2_d = nc.dram_tensor("gbv2", [128, 2], F32, kind="ExternalInput")

    attn_d = nc.dram_tensor("attn_p", [NQ, N], F32, kind="ExternalOutput")
    out_d = nc.dram_tensor("out_p", [C, NQ], F32, kind="ExternalOutput")
    out2_d = nc.dram_tensor("out2_p", [C, NQ], F32, kind="ExternalOutput")

    Exp = mybir.ActivationFunctionType.Exp
    Ident = mybir.ActivationFunctionType.Identity
    AX = mybir.AxisListType.X
    ADD = mybir.AluOpType.add

    with tile.TileContext(nc) as tc:
        with tc.tile_pool(name="persist", bufs=1) as pp:
            # ---- load weights/biases ----
            wqt = pp.tile([128, 2, CQ], F32)
            wkt = pp.tile([128, 2, CQ], F32)
            wk2t = pp.tile([128, 2, CQ], F32)
            wvt = pp.tile([128, 2, C], BF16)
            wv2t = pp.tile([128, 2, C], BF16)
            bq = pp.tile([CQ, 1], F32)
            bk = pp.tile([CQ, 1], F32)
            bk2 = pp.tile([CQ, 1], F32)
            gbv = pp.tile([128, 2], F32)
            gbv2 = pp.tile([128, 2], F32)
            nc.sync.dma_start(wqt[:], wqt_d[:])
            nc.sync.dma_start(bq[:], bq_d[:])
            for sb, d in (
                (wkt, wkt_d), (wk2t, wk2t_d),
                (wvt, wvt_d), (wv2t, wv2t_d),
                (bk, bk_d), (bk2, bk2_d),
                (gbv, gbv_d), (gbv2, gbv2_d),
            ):
                nc.gpsimd.dma_start(sb[:], d[:])

            # ---- projections ----
            # k1 in partitions 0:64, k2 in 64:128; q duplicated into both
            # partition ranges so S1/S2 matmuls have matching base partitions
            if qk_mode == "b3":
                # khh1 = [k1_hi; k1_hi], khh2 = [k2_hi; k2_hi] (dup partitions)
                # kl12 = [k1_lo; k2_lo]; qhl = [q_hi; q_lo]; qh2 = [q_hi; q_hi]
                # -> S pass A: qhl.T @ khh  (128-deep: qh*kh + ql*kh)
                #    S pass B: qh2[rows].T @ kl12[rows]  (64-deep: qh*kl)
                khh1 = pp.tile([128, N], BF16)
                khh2 = pp.tile([128, N], BF16)
                kl12 = pp.tile([128, N], BF16)
                qhl = pp.tile([128, NQ], BF16)
                qh2 = pp.tile([128, NQ], BF16)
            else:
                k12_sb = pp.tile([128, N], qkd)
                q_sb = pp.tile([128, NQ], qkd)
            r1 = pp.tile([128, 2, NQ], F32)
            r2 = pp.tile([128, 2, NQ], F32)
            vt1 = pp.tile([128, N // 128, C], BF16)
            vt2 = pp.tile([128, N // 128, C], BF16)

            phase_a = (
                tc.tile_pool(name="xload", bufs=3),
                tc.tile_pool(name="xbf", bufs=2),
                tc.tile_pool(name="pps", bufs=2, space=bass.MemorySpace.PSUM),
            )
            xp, xbp, pps = [p.__enter__() for p in phase_a]

            engs = [nc.sync, nc.gpsimd, nc.scalar]
            ecnt = [0]

            def load_x(dram, cols, eng=None):
                ts = []
                for j in range(2):
                    t = xp.tile([128, cols], F32, tag="xload")
                    h = cols // 2
                    for lo, hi in ((0, h), (h, cols)):
                        e = eng or engs[ecnt[0] % len(engs)]
                        ecnt[0] += 1
                        e.dma_start(
                            t[:, lo:hi], dram[j * 128:(j + 1) * 128, lo:hi]
                        )
                    ts.append(t)
                return ts

            def copy_kq(ps, bias, sl, out_tile, out_lo):
                # full value v = ps + bias; hi = bf16(v); lo = bf16(v - hi)
                if qk_mode == "b3":
                    nc.vector.tensor_scalar_add(out_tile[sl], ps[:], bias[:])
                    nc.vector.scalar_tensor_tensor(
                        out=out_lo[sl], in0=ps[:], scalar=bias[:],
                        in1=out_tile[sl],
                        op0=ADD, op1=mybir.AluOpType.subtract,
                    )
                else:
                    nc.vector.tensor_scalar_add(out_tile[sl], ps[:], bias[:])

            def proj_kq(xts, wt, bias, out_tile, ncols, out_lo=None):
                # out[o, n] = sum_c Wt[c, o] * x[c, n] + b[o]   (o: CQ rows)
                for fc in range(ncols // 512):
                    ps = pps.tile([CQ, 512], F32, tag="kq")
                    for j in range(2):
                        nc.tensor.matmul(
                            ps[:],
                            wt[:, j, :],
                            xts[j][:, fc * 512:(fc + 1) * 512],
                            start=(j == 0),
                            stop=(j == 1),
                        )
                    copy_kq(
                        ps, bias, np.s_[:, fc * 512:(fc + 1) * 512],
                        out_tile, out_lo,
                    )

            if qk_f32r:
                wktr = pp.tile([128, 2, CQ], F32R)
                wk2tr = pp.tile([128, 2, CQ], F32R)
                nc.vector.tensor_copy(wktr[:], wkt[:])
                nc.vector.tensor_copy(wk2tr[:], wk2t[:])
            x3qt = load_x(x3q_d, NQ)
            if qk_mode == "b3":
                ql_t = xbp.tile([CQ, NQ], BF16, tag="qlt")
                proj_kq(x3qt, wqt, bq, qhl[0:CQ, :], NQ, ql_t[:])
                for fc in range(NQ // 512):
                    cs = np.s_[fc * 512:(fc + 1) * 512]
                    nc.sync.dma_start(qh2[0:CQ, cs], qhl[0:CQ, cs])
                    nc.sync.dma_start(qh2[CQ:128, cs], qhl[0:CQ, cs])
                    nc.gpsimd.dma_start(qhl[CQ:128, cs], ql_t[:, cs])
            else:
                proj_kq(x3qt, wqt, bq, q_sb[0:CQ, :], NQ)
                for fc in range(NQ // 512):
                    nc.sync.dma_start(
                        q_sb[CQ:128, fc * 512:(fc + 1) * 512],
                        q_sb[0:CQ, fc * 512:(fc + 1) * 512],
                    )
            # residuals R = x_q + gamma*bv  (layout [p, c, n])
            for cc in range(2):
                nc.vector.tensor_scalar_add(
                    r1[:, cc, :], x3qt[cc][:], gbv[:, cc:cc + 1]
                )
            xtqt = load_x(xtq_d, NQ)
            for cc in range(2):
                nc.vector.tensor_scalar_add(
                    r2[:, cc, :], xtqt[cc][:], gbv2[:, cc:cc + 1]
                )

            def proj_k(dram, wt_r, bias, prow):
                xts = load_x(dram, N)
                if qk_f32r:
                    xr = []
                    for j in range(2):
                        t = xp.tile(
                            [128, N], F32R, tag="xr", name=f"xr{ecnt[0]}", bufs=2
                        )
                        ecnt[0] += 1
                        nc.vector.tensor_copy(t[:], xts[j][:])
                        xr.append(t)
                else:
                    xr = xts
                for fc in range(N // 512):
                    ps = pps.tile([CQ, 512], F32, tag="kq", name=f"kqp{ecnt[0]}")
                    ecnt[0] += 1
                    for j in range(2):
                        nc.tensor.matmul(
                            ps[:],
                            wt_r[:, j, :],
                            xr[j][:, fc * 512:(fc + 1) * 512],
                            start=(j == 0),
                            stop=(j == 1),
                        )
                    sl = np.s_[0:CQ, fc * 512:(fc + 1) * 512]
                    if qk_mode == "b3":
                        copy_kq(ps, bias, sl, dst_h, dst_l)
                    else:
                        sl2 = np.s_[prow:prow + CQ, fc * 512:(fc + 1) * 512]
                        nc.vector.tensor_scalar_add(k12_sb[sl2], ps[:], bias[:])

            if qk_mode == "b3":
                # hi/lo built at partition 0 (stt requires equal start
                # partitions), then DMA-duplicated/shifted into place
                k2h_t = xbp.tile([CQ, N], BF16, tag="k2t", bufs=2)
                k2l_t = xbp.tile([CQ, N], BF16, tag="k2t", bufs=2)
                dst_h, dst_l = khh1, kl12
                proj_k(x1_d, wkt, bk, 0)
                nc.sync.dma_start(khh1[CQ:128, :], khh1[0:CQ, :])
                dst_h, dst_l = k2h_t, k2l_t
                proj_k(x2_d, wk2t, bk2, 0)
                nc.sync.dma_start(khh2[0:CQ, :], k2h_t[:])
                nc.gpsimd.dma_start(khh2[CQ:128, :], k2h_t[:])
                nc.gpsimd.dma_start(kl12[CQ:128, :], k2l_t[:])
            else:
                proj_k(x1_d, wktr if qk_f32r else wkt, bk, 0)
                proj_k(x2_d, wk2tr if qk_f32r else wk2t, bk2, CQ)

            # V^T projections (bf16): vt[p, m, o] = (gamma*V)[o, 128m + p]
            def proj_vt(dram, wt, out_tile):
                xts = load_x(dram, N)
                xbf = []
                for j in range(2):
                    t = xbp.tile([128, N], BF16, tag="xbf")
                    nc.gpsimd.tensor_copy(t[:], xts[j][:])
                    xbf.append(t)
                for m in range(N // 128):
                    ps = pps.tile([128, C], F32, tag="v")
                    for j in range(2):
                        nc.tensor.matmul(
                            ps[:],
                            xbf[j][:, m * 128:(m + 1) * 128],
                            wt[:, j, :],
                            start=(j == 0),
                            stop=(j == 1),
                        )
                    if m % 2 == 0:
                        nc.vector.tensor_copy(out_tile[:, m, :], ps[:])
                    else:
                        nc.scalar.activation(
                            out=out_tile[:, m, :], in_=ps[:], func=Ident
                        )

            proj_vt(x3_d, wvt, vt1)
            proj_vt(xt_d, wv2t, vt2)
            for p in reversed(phase_a):
                p.__exit__(None, None, None)

            main_pools = (
                tc.tile_pool(name="e", bufs=5),
                tc.tile_pool(name="abf", bufs=1),
                tc.tile_pool(name="at", bufs=2),
                tc.tile_pool(name="o", bufs=2),
                tc.tile_pool(name="stat", bufs=10),
                tc.tile_pool(name="sps", bufs=3, space=bass.MemorySpace.PSUM),
                tc.tile_pool(name="avps", bufs=1, space=bass.MemorySpace.PSUM),
            )
            ep, abfp, atp, op_, stp, sps, avps = [p.__enter__() for p in main_pools]

            # ---- main loop over query blocks ----
            # AV matmuls for block i-2 are emitted in small chunks between
            # block i's S-phase quarters so PE gaps never starve ACT.
            MCH = 8  # AV k-chunks (matmuls) per emitted chunk
            av_queue = []

            def queue_av(at_t, blk):
                cols = blk * QB
                state = {}

                def mk(oi, cc, mg, vt_s, r_s, od):
                    def emit():
                        if oi not in state:
                            state[oi] = avps.tile(
                                [128, 2, QB], F32, tag=f"av{oi}",
                                name=f"avps{oi}_{blk}",
                            )
                        ps = state[oi]
                        for m in range(mg * MCH, (mg + 1) * MCH):
                            nc.tensor.matmul(
                                ps[:, cc, :],
                                vt_s[:, m, cc * 128:(cc + 1) * 128],
                                at_t[:, m, :],
                                start=(m == 0),
                                stop=(m == N // 128 - 1),
                            )
                        if cc == 1 and mg == (N // 128) // MCH - 1:
                            o_t = op_.tile([128, 2, QB], F32, tag="o")
                            for c2 in range(2):
                                nc.vector.tensor_add(
                                    o_t[:, c2, :],
                                    ps[:, c2, :],
                                    r_s[:, c2, cols:cols + QB],
                                )
                            od_r = od[:].rearrange("(c p) n -> p c n", c=2)
                            nc.sync.dma_start(
                                od_r[:, :, cols:cols + QB], o_t[:]
                            )
                    return emit

                for oi, (vt_s, r_s, od) in enumerate(
                    ((vt1, r1, out_d), (vt2, r2, out2_d))
                ):
                    for cc in range(2):
                        for mg in range((N // 128) // MCH):
                            av_queue.append(mk(oi, cc, mg, vt_s, r_s, od))

            def emit_tail_act(i, e1, e2, s12r):
                # E3 = exp(w * s12r) (in e2), row sums via accum
                s3 = stp.tile([QB, 1], F32, tag="st", name=f"s3_{i}")
                rs3 = stp.tile([QB, 1], F32, tag="st", name=f"rs3_{i}")
                nc.scalar.activation(
                    out=e2[:], in_=e1[:], func=Exp, scale=s12r[:],
                    accum_out=s3[:],
                )
                nc.vector.reciprocal(rs3[:], s3[:])
                # A = E3 * rs3 (fp32 for DRAM into e1's slot, bf16 for AV)
                nc.vector.tensor_scalar_mul(e1[:], e2[:], rs3[:])
                abf = abfp.tile([QB, N], BF16, tag="abf", name=f"abf_{i}")
                nc.gpsimd.tensor_scalar_mul(abf[:], e2[:], rs3[:])
                # transpose: at[p, m, q] = A[q, 128m + p]
                at = atp.tile([128, N // 128, QB], BF16, tag="at", name=f"at_{i}")
                nc.sync.dma_start(at[:], abf[:], transpose=True)
                # attn store split across SP and SWDGE queues
                rows = attn_d[i * QB:(i + 1) * QB, :]
                nc.sync.dma_start(rows[:, 0:N // 2], e1[:, 0:N // 2])
                nc.gpsimd.dma_start(rows[:, N // 2:N], e1[:, N // 2:N])
                done.append((at, i))

            done = []
            tails = []
            for i in range(NBLK):
                if len(done) >= 3 or (i >= NBLK - 2 and len(done) >= 2):
                    queue_av(*done.pop(0))
                e1 = ep.tile([QB, N], F32, tag="e", name=f"e1_{i}")
                e2 = ep.tile([QB, N], F32, tag="e", name=f"e2_{i}")
                s1p = stp.tile([QB, 4], F32, tag="sp", name=f"s1p_{i}")
                s2p = stp.tile([QB, 4], F32, tag="sp", name=f"s2p_{i}")
                for srow, e_t, sp in ((0, e1, s1p), (CQ, e2, s2p)):
                    khh_s = khh1 if (qk_mode == "b3" and srow == 0) else (
                        khh2 if qk_mode == "b3" else None
                    )
                    for qt in range(4):
                        ps = sps.tile([QB, 1024], F32, tag="s", name=f"sps_{i}")
                        qsl = np.s_[srow:srow + CQ, i * QB:(i + 1) * QB]
                        if qk_mode == "b3":
                            for hf in range(2):
                                c0 = qt * 1024 + hf * 512
                                psl = np.s_[:, hf * 512:(hf + 1) * 512]
                                nc.tensor.matmul(
                                    ps[psl],
                                    qhl[:, i * QB:(i + 1) * QB],
                                    khh_s[:, c0:c0 + 512],
                                    start=True, stop=False,
                                )
                                nc.tensor.matmul(
                                    ps[psl],
                                    qh2[qsl],
                                    kl12[srow:srow + CQ, c0:c0 + 512],
                                    start=False, stop=True,
                                )
                        else:
                            for hf in range(2):
                                c0 = qt * 1024 + hf * 512
                                nc.tensor.matmul(
                                    ps[:, hf * 512:(hf + 1) * 512],
                                    q_sb[qsl],
                                    k12_sb[srow:srow + CQ, c0:c0 + 512],
                                    start=True,
                                    stop=True,
                                )
                        nc.scalar.activation(
                            out=e_t[:, qt * 1024:(qt + 1) * 1024],
                            in_=ps[:],
                            func=Exp,
                            accum_out=sp[:, qt:qt + 1],
                        )
                        for _ in range(2 if i < NBLK - 4 else 3):
                            if av_queue:
                                av_queue.pop(0)()
                # software pipeline: this block's DVE chain inline; the
                # exp3-and-later part is emitted after the NEXT block's
                # S-phase so ACT's in-order stream never waits on DVE.
                if tails:
                    tails.pop(0)()
                s1 = stp.tile([QB, 1], F32, tag="st", name=f"s1_{i}")
                s2 = stp.tile([QB, 1], F32, tag="st", name=f"s2_{i}")
                s12r = stp.tile([QB, 1], F32, tag="st", name=f"s12r_{i}")
                nc.vector.tensor_reduce(out=s1[:], in_=s1p[:], axis=AX, op=ADD)
                nc.vector.tensor_reduce(out=s2[:], in_=s2p[:], axis=AX, op=ADD)
                s12 = stp.tile([QB, 1], F32, tag="st", name=f"s12_{i}")
                nc.vector.tensor_mul(s12[:], s1[:], s2[:])
                nc.vector.reciprocal(s12r[:], s12[:])
                # w2 = E2*s1 (in e2); w = E1*s2 + w2 (in e1)
                nc.vector.tensor_scalar_mul(e2[:], e2[:], s1[:])
                nc.vector.scalar_tensor_tensor(
                    out=e1[:], in0=e1[:], scalar=s2[:], in1=e2[:],
                    op0=mybir.AluOpType.mult, op1=ADD,
                )
                tails.append(
                    lambda i=i, e1=e1, e2=e2, s12r=s12r: emit_tail_act(
                        i, e1, e2, s12r
                    )
                )

            while tails:
                tails.pop(0)()
            # drain the tail: AV for the last blocks
            while done or av_queue:
                if av_queue:
                    av_queue.pop(0)()
                elif done:
                    queue_av(*done.pop(0))

            for p in reversed(main_pools):
                p.__exit__(None, None, None)

    nc.compile()
    return nc


def shard_inputs(inputs):
    """Full inputs -> per-core in_maps."""
    f32 = np.float32
    x1 = np.ascontiguousarray(inputs["x1"], f32).reshape(B, C, N)
    x2 = np.ascontiguousarray(inputs["x2"], f32).reshape(B, C, N)
    x3 = np.ascontiguousarray(inputs["x3"], f32).reshape(B, C, N)
    xt = np.ascontiguousarray(inputs["xt"], f32).reshape(B, C, N)
    gamma = f32(np.asarray(inputs["gamma"]).reshape(())[()])
    gamma2 = f32(np.asarray(inputs["gamma2"]).reshape(())[()])

    def wt_small(w):  # (CQ, C) -> (128, 2, CQ)
        w = np.asarray(w, f32)
        return np.ascontiguousarray(w.T.reshape(2, 128, CQ).transpose(1, 0, 2))

    def wt_big(w, g):  # (C, C) -> (128, 2, C) bf16, gamma-folded
        w = np.asarray(w, np.float64) * np.float64(g)
        t = w.T.reshape(2, 128, C).transpose(1, 0, 2)
        return np.ascontiguousarray(t.astype(ml_dtypes.bfloat16))

    shared = {
        "wqt": wt_small(inputs["Wq"]),
        "wkt": wt_small(inputs["Wk"]),
        "wk2t": wt_small(inputs["Wk2"]),
        "wvt": wt_big(inputs["Wv"], gamma),
        "wv2t": wt_big(inputs["Wv2"], gamma2),
        "bq": np.ascontiguousarray(np.asarray(inputs["bq"], f32).reshape(CQ, 1)),
        "bk": np.ascontiguousarray(np.asarray(inputs["bk"], f32).reshape(CQ, 1)),
        "bk2": np.ascontiguousarray(np.asarray(inputs["bk2"], f32).reshape(CQ, 1)),
        "gbv": np.ascontiguousarray(
            (gamma * np.asarray(inputs["bv"], f32)).reshape(2, 128).T
        ),
        "gbv2": np.ascontiguousarray(
            (gamma2 * np.asarray(inputs["bv2"], f32)).reshape(2, 128).T
        ),
    }
    in_maps = []
    for core in range(N_CORES):
        b, h = core // 2, core % 2
        sl = slice(h * NQ, (h + 1) * NQ)
        in_maps.append(
            {
                "x1": np.ascontiguousarray(x1[b]),
                "x2": np.ascontiguousarray(x2[b]),
                "x3": np.ascontiguousarray(x3[b]),
                "xt": np.ascontiguousarray(xt[b]),
                "x3q": np.ascontiguousarray(x3[b][:, sl]),
                "xtq": np.ascontiguousarray(xt[b][:, sl]),
                **shared,
            }
        )
    return in_maps


def gather_outputs(results):
    attn = np.empty((B, N, N), np.float32)
    out = np.empty((B, C, N), np.float32)
    out2 = np.empty((B, C, N), np.float32)
    for core in range(N_CORES):
        b, h = core // 2, core % 2
        sl = slice(h * NQ, (h + 1) * NQ)
        r = results[core]
        attn[b, sl, :] = r["attn_p"]
        out[b][:, sl] = r["out_p"]
        out2[b][:, sl] = r["out2_p"]
    return (
        attn,
        out.reshape(B, C, HH, WW),
        out2.reshape(B, C, HH, WW),
    )


_NC_CACHE = {}


def _get_nc(qk_mode: str = QK_MODE):
    if qk_mode not in _NC_CACHE:
        _NC_CACHE[qk_mode] = build_nc(qk_mode)
    return _NC_CACHE[qk_mode]


def kernel(**inputs):
    nc = _get_nc()
    in_maps = shard_inputs(inputs)
    res = run_bass_kernel_spmd(nc, in_maps, list(range(N_CORES)))
    return gather_outputs(res.results)
